# revision 3
# baseline (speedup 1.0000x reference)
"""Trainium2 Bass kernel for nn_Attention (B=4, N=2048, C=1024, H=16, D=64).

v3 — restructured from baseline for engine balance / overlap:
  - qkv q/k projection: single PSUM pass (8-kt accumulation) per
    [128,512] token chunk; bias fused in the DVE drain (tensor_scalar).
  - Dedicated PSUM pools: sc (2x[128,1024] = 4 banks, double-buffered
    scores), po (2x[65,512] = 2 banks, PV accumulators), work
    (2x[128,512] = 2 banks, everything else).
  - RMSNorm rsqrt without ACT Ln (kills table-set thrash): DVE magic-log
    seed + two exp-Newton refinements, all in the Exp table set.
  - q and k sumsq packed into one [96,N] stats tile (zero-padded BLK
    columns keep both matmuls writing the same PSUM region).
  - 2 of 16 jt exp tiles per chunk on DVE via int16 magic-exp (bitcast
    to bf16 for the PV matmul) to relieve the ACT-paced softmax loop.
  - Pipelined emission with fillers: qkv/stats/proj matmul groups are
    interleaved into the attention jt stream at jt 4/8/12 boundaries.
  - Pair 3: per-ic attention_end + interleaved output projection.
  - sq (squares for RMSNorm stats) on the idle Pool/GpSimd engine.
  - DMA order: wq/wk + xtb first, WP last; weight loads merged into
    single rearranged DMAs.

Sharding: 8 cores; core c handles batch b=c//2 and heads [8*(c%2), ...).
Host sums the two partial proj outputs per batch.
"""

import os
import numpy as np
import ml_dtypes

B, N, C, H, D = 4, 2048, 1024, 16, 64
NCORES = 8
HPC = 8           # heads per core
CH = HPC * D      # 512 channels per core
VSEG = 2 * D + 4  # 132 cols per pair in v_aug: [64 v | 1 | 1][64 v | 1 | 1]
VW = 4 * VSEG     # 528
EPS = 1e-6

_CACHE = {}
LAST_RESULT = [None]


def _round_f32r(x):
    x = np.ascontiguousarray(x, dtype=np.float32)
    u = x.view(np.uint32)
    keep = np.uint32(0xFFFFF000)
    half = np.uint32(0x800)
    lsb = (u >> np.uint32(12)) & np.uint32(1)
    r = (u + (half - np.uint32(1)) + lsb) & keep
    return r.view(np.float32)


def _build_nc():
    import concourse.tile as tile
    import concourse.mybir as mybir
    from concourse import bacc

    F32 = mybir.dt.float32
    F32R = mybir.dt.float32r
    BF16 = mybir.dt.bfloat16
    AF = mybir.ActivationFunctionType

    nc = bacc.Bacc("TRN2", target_bir_lowering=False, debug=False,
                   num_devices=NCORES)

    XTB = nc.dram_tensor("XTB", [C, N], BF16, kind="ExternalInput")
    WQ = nc.dram_tensor("WQ", [C, CH], BF16, kind="ExternalInput")
    WK = nc.dram_tensor("WK", [C, CH], BF16, kind="ExternalInput")
    WVA = nc.dram_tensor("WVA", [C, VW], BF16, kind="ExternalInput")
    WP = nc.dram_tensor("WP", [CH, C], F32R, kind="ExternalInput")
    BQK = nc.dram_tensor("BQK", [128, 8], F32, kind="ExternalInput")
    BVA = nc.dram_tensor("BVA", [128, VW], F32, kind="ExternalInput")
    BP = nc.dram_tensor("BP", [128, C], F32, kind="ExternalInput")
    QKN = nc.dram_tensor("QKN", [128, 2], F32, kind="ExternalInput")
    BLK = nc.dram_tensor("BLK", [128, 192], F32R, kind="ExternalInput")
    SEL = nc.dram_tensor("SEL", [96, 256], F32R, kind="ExternalInput")
    ONESB = nc.dram_tensor("ONESB", [65, 64], F32R, kind="ExternalInput")
    Y = nc.dram_tensor("Y", [N, C], F32, kind="ExternalOutput")

    NT = N // 128          # 16 token tiles
    KT = C // 128          # 8 contraction tiles
    NCHUNK = N // 512      # 4 chunks of 512 tokens

    with tile.TileContext(nc) as tc:
        from contextlib import ExitStack
        with ExitStack() as ctx:
            const_p = ctx.enter_context(tc.tile_pool(name="const", bufs=1))
            xtb_p = ctx.enter_context(tc.tile_pool(name="xtb", bufs=8))
            outT_p = ctx.enter_context(tc.tile_pool(name="outT", bufs=4))

            # PSUM pools: sc 4 banks, po 2 banks, work 2 banks
            sc_ps = ctx.enter_context(
                tc.tile_pool(name="sc", bufs=2, space="PSUM"))
            po_ps = ctx.enter_context(
                tc.tile_pool(name="po", bufs=2, space="PSUM"))
            work_ps = ctx.enter_context(
                tc.tile_pool(name="work", bufs=2, space="PSUM"))

            # ---- weights for pair 0 first, then x, then constants ----
            pair_ctx = ExitStack()
            w_p = ctx.enter_context(tc.tile_pool(name="w", bufs=2))
            state = {}

            def emit_w_loads(hp, with_xtb=False):
                wq_sb = w_p.tile([128, C], BF16, tag="wq")
                wk_sb = w_p.tile([128, C], BF16, tag="wk")
                wq3 = WQ.ap().rearrange("(kt p) j -> p kt j", p=128)
                wk3 = WK.ap().rearrange("(kt p) j -> p kt j", p=128)
                hsl = slice(hp * 128, (hp + 1) * 128)
                nc.sync.dma_start(wq_sb[:], wq3[:, :, hsl])
                nc.sync.dma_start(wk_sb[:], wk3[:, :, hsl])
                if with_xtb:
                    for kt in range(KT):
                        nc.sync.dma_start(
                            xtb_tiles[kt][:],
                            XTB.ap()[kt * 128:(kt + 1) * 128, :])
                st = state[hp] = {}
                st["wq"], st["wk"] = wq_sb, wk_sb
                st["qT_b"] = qtb_p.tile([128, N], F32, tag="qtb",
                                        name=f"qTb{hp}")
                st["kT_b"] = qtb_p.tile([128, N], F32, tag="ktb",
                                        name=f"kTb{hp}")

            qtb_p = ctx.enter_context(tc.tile_pool(name="qtb", bufs=1))

            xtb_tiles = []
            for kt in range(KT):
                xtb_tiles.append(
                    xtb_p.tile([128, N], BF16, tag="xtb", name=f"xtb{kt}"))
            emit_w_loads(0, with_xtb=True)

            # constants
            bqk_sb = const_p.tile([128, 8], F32, tag="bqk")
            nc.sync.dma_start(bqk_sb[:], BQK.ap()[:, :])
            bva_sb = const_p.tile([128, VW], F32, tag="bva")
            nc.sync.dma_start(bva_sb[:], BVA.ap()[:, :])
            qkn_sb = const_p.tile([128, 2], F32, tag="qkn")
            nc.sync.dma_start(qkn_sb[:], QKN.ap()[:, :])
            blk_sb = const_p.tile([128, 192], F32R, tag="blk")
            nc.sync.dma_start(blk_sb[:], BLK.ap()[:, :])
            sel_sb = const_p.tile([96, 256], F32R, tag="sel")
            nc.sync.dma_start(sel_sb[:], SEL.ap()[:, :])
            onesb_sb = const_p.tile([65, 64], F32R, tag="onesb")
            nc.sync.dma_start(onesb_sb[:], ONESB.ap()[:, :])
            eps_qk = const_p.tile([128, 1], F32, tag="eps_qk")
            nc.vector.memset(eps_qk[0:64, :], float(EPS))
            nc.vector.memset(eps_qk[64:128, :], float(EPS * 64))

            # ---------------- qkv q/k projection ------------------------
            def emit_qk_proj(hp, which, chunks=None):
                # which: 0 = q, 1 = k; single psum pass over all 8 kt
                st = state[hp]
                wsb = st["wk"] if which else st["wq"]
                dst = st["kT_b"] if which else st["qT_b"]
                bcol = (4 + hp) if which else hp
                for chk in (chunks if chunks is not None
                            else range(NCHUNK)):
                    csl = slice(chk * 512, (chk + 1) * 512)
                    ps = work_ps.tile([128, 512], F32, tag="work")
                    for kt in range(KT):
                        nc.tensor.matmul(
                            ps[:], wsb[:, kt * 128:(kt + 1) * 128],
                            xtb_tiles[kt][:, csl],
                            start=(kt == 0), stop=(kt == KT - 1))
                    nc.vector.tensor_scalar(
                        dst[:, csl], ps[:], bqk_sb[:, bcol:bcol + 1],
                        None, op0=mybir.AluOpType.add)

            # ---------------- stats (RMSNorm rsqrt) ---------------------
            def emit_stats_front(hp, chunks=None):
                # q var lands at rows 0/32, k sumsq at rows 64/96 of one
                # packed [97, N] tile (two matmuls, second accumulates into
                # the same psum bank at col position 64).  rsqrt is computed
                # without Ln (avoids ACT table switching amid the exps):
                # magic-log bit trick seeds x0 ~ ln(v), two Newton steps
                # x <- x - 1 + v*exp(-x) refine it on DVE+ACT(Exp), then
                # rs = exp(-0.5 x).
                st = state[hp]
                I32 = mybir.dt.int32
                if "rs" not in st:
                    st["rs"] = stat_p.tile([96, N], F32R, tag="rs",
                                           name=f"rs{hp}")
                rs = st["rs"]
                for chk in (chunks if chunks is not None
                            else range(NCHUNK)):
                    csl = slice(chk * 512, (chk + 1) * 512)
                    sq_q = sq_p.tile([128, 512], F32R, tag="sq")
                    nc.gpsimd.tensor_mul(sq_q[:], st["qT_b"][:, csl],
                                         st["qT_b"][:, csl])
                    sq_k = sq_p.tile([128, 512], F32R, tag="sq")
                    nc.gpsimd.tensor_mul(sq_k[:], st["kT_b"][:, csl],
                                         st["kT_b"][:, csl])
                    vps = work_ps.tile([96, 512], F32, tag="work")
                    nc.tensor.matmul(vps[:, :], blk_sb[:, 0:96], sq_q[:],
                                     start=True, stop=False)
                    nc.tensor.matmul(vps[:, :], blk_sb[:, 96:192],
                                     sq_k[:], start=False, stop=True)
                    vv = stat_p.tile([96, 512], F32, tag="vv")
                    nc.vector.tensor_scalar(vv[:], vps[:],
                                            eps_qk[0:96, :], None,
                                            op0=mybir.AluOpType.add)
                    x0 = stat_p.tile([96, 512], BF16, tag="x0")
                    nc.vector.tensor_scalar(
                        x0[:], vv[:].bitcast(I32), 8.2629582e-8,
                        -87.989971088, op0=mybir.AluOpType.mult,
                        op1=mybir.AluOpType.add)
                    e0 = stat_p.tile([96, 512], F32, tag="e")
                    nc.scalar.activation(e0[:], x0[:], AF.Exp, scale=-1.0)
                    t0 = stat_p.tile([96, 512], F32, tag="t")
                    nc.vector.tensor_mul(t0[:], vv[:], e0[:])
                    x1 = stat_p.tile([96, 512], F32, tag="xf", bufs=2)
                    nc.vector.scalar_tensor_tensor(
                        x1[:], t0[:], -1.0, x0[:],
                        op0=mybir.AluOpType.add, op1=mybir.AluOpType.add)
                    e1 = stat_p.tile([96, 512], F32, tag="e")
                    nc.scalar.activation(e1[:], x1[:], AF.Exp, scale=-1.0)
                    t1 = stat_p.tile([96, 512], F32, tag="t")
                    nc.vector.tensor_mul(t1[:], vv[:], e1[:])
                    x2 = stat_p.tile([96, 512], F32, tag="xf", bufs=2)
                    nc.vector.scalar_tensor_tensor(
                        x2[:], t1[:], -1.0, x1[:],
                        op0=mybir.AluOpType.add, op1=mybir.AluOpType.add)
                    nc.scalar.activation(rs[:, csl], x2[:], AF.Exp,
                                         scale=-0.5)

            def emit_stats_apply(hp, which=None):
                st = state[hp]
                rs = st["rs"]
                if "qTn" not in st:
                    st["qTn"] = qtn_p.tile([128, N], BF16, tag="qtn",
                                           name=f"qTn{hp}")
                    st["kTn"] = qtn_p.tile([128, N], BF16, tag="ktn",
                                           name=f"kTn{hp}")
                todo = ((0, 1) if which is None else (which,))
                for w in todo:
                    src_t = st["kT_b"] if w else st["qT_b"]
                    dstn = st["kTn"] if w else st["qTn"]
                    selsl = slice(128, 256) if w else slice(0, 128)
                    for chk in range(NCHUNK):
                        csl = slice(chk * 512, (chk + 1) * 512)
                        bc_ps = work_ps.tile([128, 512], F32, tag="work",
                                             name="bc_ps")
                        nc.tensor.matmul(
                            bc_ps[:], sel_sb[:, selsl], rs[:, csl],
                            start=True, stop=True)
                        nc.vector.scalar_tensor_tensor(
                            dstn[:, csl], src_t[:, csl],
                            qkn_sb[:, w:w + 1], bc_ps[:],
                            op0=mybir.AluOpType.mult,
                            op1=mybir.AluOpType.mult)

            # ---------------- v projection ------------------------------
            v_ctx = tc.tile_pool(name="v", bufs=16)
            v_p = v_ctx.__enter__()
            v_tiles = []
            for nt in range(NT):
                v_tiles.append(
                    v_p.tile([128, VW], BF16, tag="v", name=f"vt{nt}"))

            def emit_v_block(nt_range):
                for nt in nt_range:
                    for vh in range(4):
                        sl = slice(vh * (VW // 4), (vh + 1) * (VW // 4))
                        ps = work_ps.tile([128, VW // 4], F32, tag="work")
                        for kt in range(KT):
                            nc.tensor.matmul(
                                ps[:],
                                xtb_tiles[kt][:, nt * 128:(nt + 1) * 128],
                                wv_sb[:, kt * VW + vh * (VW // 4):
                                      kt * VW + (vh + 1) * (VW // 4)],
                                start=(kt == 0), stop=(kt == KT - 1))
                        nc.vector.tensor_add(
                            v_tiles[nt][:, sl], ps[:], bva_sb[:, sl])

            # ---------------- attention -------------------------------
            outT_tiles = []

            def emit_attention_start(hp):
                st = state[hp]
                outT = outT_p.tile([128, N], F32R, tag="outT",
                                   name=f"outT{hp}")
                outT_tiles.append(outT)
                st["outT"] = outT
                if hp < 3:
                    st["den"] = rcp_p.tile([8, 512], F32, tag="den_pack",
                                           bufs=1, name=f"den{hp}")
                st["po_sbs"] = []

            # int16 magic-exp constants: exp(s) ~ bf16_bits(s*A2 + B2);
            # B2 midpoint is robust to floor-vs-round f32->i16 conversion.
            MEXP_A = 128.0 / float(np.log(2.0))
            MEXP_B = 16250.75
            DVE_EXP_JTS = (7, 15)

            def emit_attention_ic(hp, ic, fillers=()):
                # fillers: up to 4 emit-thunks interleaved into the jt
                # stream (at jt 4/8/12 and after the last jt) so PE work
                # like qkv/stats/proj overlaps the ACT-paced exp stream.
                st = state[hp]
                qTn, kTn, outT = st["qTn"], st["kTn"], st["outT"]
                I16 = mybir.dt.int16
                vbase = hp * VSEG
                isl = slice(ic * 512, (ic + 1) * 512)
                poA = po_ps.tile([65, 512], F32, tag="po")
                poB = po_ps.tile([65, 512], F32, tag="po")
                for jt in range(NT):
                    if jt in (4, 8, 12) and len(fillers) > jt // 4 - 1:
                        fillers[jt // 4 - 1]()
                    jsl = slice(jt * 128, (jt + 1) * 128)
                    sc = sc_ps.tile([128, 1024], F32, tag="sc")
                    nc.tensor.matmul(
                        sc[:, 0:512], kTn[0:64, jsl], qTn[0:64, isl],
                        start=True, stop=True, tile_position=(0, 0))
                    nc.tensor.matmul(
                        sc[:, 512:1024], kTn[64:128, jsl], qTn[64:128, isl],
                        start=True, stop=True, tile_position=(64, 0))
                    if jt in DVE_EXP_JTS:
                        exi = ex_p.tile([128, 1024], I16, tag="ex")
                        with nc.allow_low_precision(
                                reason="magic-exp approx on DVE"):
                            nc.vector.tensor_scalar(
                                exi[:], sc[:], MEXP_A, MEXP_B,
                                op0=mybir.AluOpType.mult,
                                op1=mybir.AluOpType.add)
                        ex0 = exi[:, 0:512].bitcast(BF16)
                        ex1 = exi[:, 512:1024].bitcast(BF16)
                    else:
                        ext = ex_p.tile([128, 1024], BF16, tag="ex")
                        nc.scalar.activation(ext[:], sc[:], AF.Exp)
                        ex0 = ext[:, 0:512]
                        ex1 = ext[:, 512:1024]
                    nc.tensor.matmul(
                        poA[:], v_tiles[jt][:, vbase:vbase + 65],
                        ex0, start=(jt == 0), stop=(jt == NT - 1))
                    nc.tensor.matmul(
                        poB[:],
                        v_tiles[jt][:, vbase + VSEG // 2:
                                    vbase + VSEG // 2 + 65],
                        ex1, start=(jt == 0),
                        stop=(jt == NT - 1))
                if len(fillers) > 3:
                    fillers[3]()
                for hh, (po, rowoff) in enumerate(((poA, 0), (poB, 64))):
                    idx = ic * 2 + hh
                    po_sb = rcp_p.tile([65, 512], F32, tag="po_sb",
                                       name=f"po_sb{hp}_{idx}", bufs=10)
                    nc.vector.tensor_copy(po_sb[:], po[:, :])
                    if hp < 3:
                        nc.sync.dma_start(st["den"][idx:idx + 1, :],
                                          po_sb[64:65, :])
                    st["po_sbs"].append((po_sb, rowoff, ic, idx))

            def emit_attention_end(hp):
                # pairs 0-2: batched rcp for all 4 ics
                st = state[hp]
                outT = st["outT"]
                rcp_pack = rcp_p.tile([8, 512], F32R, tag="rcp_pack",
                                      bufs=1, name=f"rcpp{hp}")
                with nc.allow_low_precision(
                        reason="softmax denom recip rounded to f32r"):
                    nc.vector.reciprocal(rcp_pack[:], st["den"][:])
                rcp_al = rcp_p.tile([65, 1536], F32R, tag="rcp_al",
                                    bufs=1, name=f"rcpa{hp}")
                for idx in range(8):
                    r, fb = 32 * (idx % 3), 512 * (idx // 3)
                    nc.sync.dma_start(rcp_al[r:r + 1, fb:fb + 512],
                                      rcp_pack[idx:idx + 1, :])
                for (po_sb, rowoff, ic, idx) in st["po_sbs"]:
                    isl = slice(ic * 512, (ic + 1) * 512)
                    r, fb = 32 * (idx % 3), 512 * (idx // 3)
                    rb_ps = work_ps.tile([64, 512], F32, tag="work",
                                         name="rb_ps")
                    nc.tensor.matmul(rb_ps[:], onesb_sb[r:r + 1, :],
                                     rcp_al[r:r + 1, fb:fb + 512],
                                     start=True, stop=True)
                    nc.vector.tensor_mul(
                        outT[rowoff:rowoff + 64, isl], po_sb[0:64, :],
                        rb_ps[:])

            def emit_attention_end3_rcp(ic):
                # pair 3, one ic: rcp + outT for its two po tiles.
                st = state[3]
                outT = st["outT"]
                den3 = rcp_p.tile([33, 512], F32, tag="den3", bufs=1,
                                  name=f"den3_{ic}")
                ents = [e for e in st["po_sbs"] if e[2] == ic]
                for (po_sb, rowoff, _ic, idx) in ents:
                    r = 32 * (idx % 2)
                    nc.sync.dma_start(den3[r:r + 1, :], po_sb[64:65, :])
                rcp3 = rcp_p.tile([33, 512], F32R, tag="rcp3", bufs=1,
                                  name=f"rcp3_{ic}")
                with nc.allow_low_precision(
                        reason="softmax denom recip rounded to f32r"):
                    nc.vector.reciprocal(rcp3[0:1, :], den3[0:1, :])
                    nc.vector.reciprocal(rcp3[32:33, :], den3[32:33, :])
                isl = slice(ic * 512, (ic + 1) * 512)
                for (po_sb, rowoff, _ic, idx) in ents:
                    r = 32 * (idx % 2)
                    rb_ps = work_ps.tile([64, 512], F32, tag="work",
                                         name="rb_ps")
                    nc.tensor.matmul(rb_ps[:], onesb_sb[r:r + 1, :],
                                     rcp3[r:r + 1, :],
                                     start=True, stop=True)
                    nc.vector.tensor_mul(
                        outT[rowoff:rowoff + 64, isl], po_sb[0:64, :],
                        rb_ps[:])

            # ---------------- output projection -------------------------
            def emit_proj_nt(nt):
                nsl = slice(nt * 128, (nt + 1) * 128)
                for sub in range(2):
                    ssl = slice(sub * 512, (sub + 1) * 512)
                    ps = work_ps.tile([128, 512], F32, tag="work")
                    for kt in range(4):
                        nc.tensor.matmul(
                            ps[:], outT_tiles[kt][:, nsl],
                            wp_tiles[kt][:, ssl],
                            start=(kt == 0), stop=(kt == 3))
                    y_sb = y_p.tile([128, 512], F32, tag="y")
                    nc.vector.tensor_add(y_sb[:], ps[:], bp_sb[:, ssl])
                    nc.sync.dma_start(Y.ap()[nsl, ssl], y_sb[:])

            # ---------------- emission schedule -------------------------
            sq_p = pair_ctx.enter_context(tc.tile_pool(name="sq", bufs=1))
            stat_p = pair_ctx.enter_context(tc.tile_pool(name="stat",
                                                         bufs=1))
            qtn_p = pair_ctx.enter_context(tc.tile_pool(name="qtn", bufs=2))
            rcp_p = pair_ctx.enter_context(tc.tile_pool(name="rcp", bufs=1))
            ex_p = pair_ctx.enter_context(tc.tile_pool(name="ex", bufs=2))
            bp_sb = const_p.tile([128, C], F32, tag="bp")
            nc.sync.dma_start(bp_sb[:], BP.ap()[:, :])

            wv_ctx = tc.tile_pool(name="wv", bufs=1)
            wv_p = wv_ctx.__enter__()
            wv_sb = wv_p.tile([128, KT * VW], BF16, tag="wv")
            nc.sync.dma_start(
                wv_sb[:],
                WVA.ap().rearrange("(kt p) j -> p kt j", p=128))

            for chk in range(NCHUNK):
                emit_qk_proj(0, 0, [chk])
                emit_qk_proj(0, 1, [chk])
                emit_stats_front(0, [chk])
                emit_v_block(range(4 * chk, 4 * chk + 4))
            emit_stats_apply(0)
            wv_ctx.__exit__(None, None, None)

            wp_tiles = []

            def emit_wp_loads():
                wp_p = pair_ctx.enter_context(tc.tile_pool(name="wp",
                                                           bufs=4))
                for kt in range(4):
                    t = wp_p.tile([128, C], F32R, tag="wp")
                    nc.sync.dma_start(t[:], WP.ap()[kt * 128:(kt + 1) * 128, :])
                    wp_tiles.append(t)

            for hp in range(4):
                emit_attention_start(hp)
                if hp + 1 < 4:
                    emit_w_loads(hp + 1)
                if hp == 2:
                    emit_wp_loads()
                    y_p = pair_ctx.enter_context(tc.tile_pool(name="y",
                                                              bufs=2))
                for ic in range(NCHUNK):
                    if hp + 1 < 4:
                        nx = hp + 1
                        if ic == 0:
                            fillers = [
                                (lambda c=c, nx=nx:
                                 emit_qk_proj(nx, 0, [c]))
                                for c in range(4)]
                            if hp > 0:
                                fillers[3] = (
                                    lambda f=fillers[3], p=hp - 1:
                                    (f(), emit_attention_end(p)))
                        elif ic == 1:
                            fillers = [
                                (lambda c=c, nx=nx:
                                 emit_qk_proj(nx, 1, [c]))
                                for c in range(4)]
                        elif ic == 2:
                            fillers = [
                                (lambda c=c, nx=nx:
                                 emit_stats_front(nx, [c]))
                                for c in range(4)]
                        else:
                            noop = lambda: None
                            fillers = [
                                noop, noop,
                                (lambda nx=nx: emit_stats_apply(nx, 0)),
                                (lambda nx=nx: emit_stats_apply(nx, 1))]
                    else:
                        if ic == 0:
                            fillers = [lambda: emit_attention_end(2)]
                        else:
                            pic = ic - 1
                            fillers = [
                                (lambda p=pic:
                                 emit_attention_end3_rcp(p)),
                                (lambda p=pic: emit_proj_nt(4 * p)),
                                (lambda p=pic: (emit_proj_nt(4 * p + 1),
                                                emit_proj_nt(4 * p + 2))),
                                (lambda p=pic: emit_proj_nt(4 * p + 3))]
                    emit_attention_ic(hp, ic, fillers)
            emit_attention_end3_rcp(3)
            for nt in range(12, 16):
                emit_proj_nt(nt)

            pair_ctx.close()
            v_ctx.__exit__(None, None, None)

    nc.compile()
    return nc


def _core_inputs(c, x, W_qkv, b_qkv, W_proj, b_proj, qn_w, kn_w):
    b, half = c // 2, c % 2
    hbase = HPC * half
    co = hbase * D                      # channel offset of this core's heads

    xT = np.ascontiguousarray(x[b].T, dtype=np.float32)
    WQc = W_qkv[:, co:co + CH].astype(ml_dtypes.bfloat16)
    WKc = W_qkv[:, C + co:C + co + CH].astype(ml_dtypes.bfloat16)
    WVc = W_qkv[:, 2 * C + co:2 * C + co + CH]
    WVA = np.zeros((C, VW), dtype=np.float32)
    BVA1 = np.zeros((VW,), dtype=np.float32)
    bv = b_qkv[2 * C + co:2 * C + co + CH]
    for hp in range(4):
        for hh in range(2):
            s = hp * VSEG + hh * (VSEG // 2)
            WVA[:, s:s + D] = WVc[:, (2 * hp + hh) * D:(2 * hp + hh + 1) * D]
            BVA1[s:s + D] = bv[(2 * hp + hh) * D:(2 * hp + hh + 1) * D]
            BVA1[s + D] = 1.0  # ones column for softmax denominators
    WVA = WVA.astype(ml_dtypes.bfloat16)
    BVA = np.broadcast_to(BVA1, (128, VW)).copy()

    BQK = np.zeros((128, 8), dtype=np.float32)
    for hp in range(4):
        BQK[:, hp] = b_qkv[co + hp * 128:co + (hp + 1) * 128]
        BQK[:, 4 + hp] = b_qkv[C + co + hp * 128:C + co + (hp + 1) * 128]

    WPc = _round_f32r(W_proj[co:co + CH, :])
    BP = (np.broadcast_to(b_proj, (128, C)).copy() if half == 0
          else np.zeros((128, C), dtype=np.float32))
    QKN = np.stack([np.tile(qn_w, 2), np.tile(kn_w, 2)], axis=1).astype(np.float32)
    BLK = np.zeros((128, 192), dtype=np.float32)
    BLK[0:64, 0] = 1.0 / D        # q head0 -> var row 0
    BLK[64:128, 32] = 1.0 / D     # q head1 -> var row 32
    BLK[0:64, 96 + 64] = 1.0      # k head0 -> row 64 (1/8 folds into rsqrt)
    BLK[64:128, 96 + 95] = 1.0    # k head1 -> row 95
    BLK = _round_f32r(BLK)
    SEL = np.zeros((96, 256), dtype=np.float32)
    SEL[0, 0:64] = 1.0      # q head0 -> channels 0-63
    SEL[32, 64:128] = 1.0   # q head1
    SEL[64, 128:192] = 1.0  # k head0
    SEL[95, 192:256] = 1.0  # k head1
    SEL = _round_f32r(SEL)
    ONESB = np.zeros((65, 64), dtype=np.float32)
    for r in (0, 32, 64):
        ONESB[r, :] = 1.0
    ONESB = _round_f32r(ONESB)

    xTb = xT.astype(ml_dtypes.bfloat16)
    return {"XTB": xTb, "WQ": WQc, "WK": WKc, "WVA": WVA, "WP": WPc,
            "BQK": BQK, "BVA": BVA, "BP": BP.astype(np.float32),
            "QKN": QKN, "BLK": BLK, "SEL": SEL, "ONESB": ONESB}


def kernel(x, W_qkv, b_qkv, W_proj, b_proj, qn_w, kn_w):
    from concourse.bass_utils import run_bass_kernel_spmd

    if "nc" not in _CACHE:
        _CACHE["nc"] = _build_nc()
    nc = _CACHE["nc"]

    args = (np.asarray(x, np.float32), np.asarray(W_qkv, np.float32),
            np.asarray(b_qkv, np.float32), np.asarray(W_proj, np.float32),
            np.asarray(b_proj, np.float32), np.asarray(qn_w, np.float32),
            np.asarray(kn_w, np.float32))
    in_maps = [_core_inputs(c, *args) for c in range(NCORES)]

    trace = os.environ.get("BASS_KERNEL_TRACE", "0") == "1"
    res = run_bass_kernel_spmd(nc, in_maps, core_ids=list(range(NCORES)),
                               trace=trace)
    LAST_RESULT[0] = res

    y = np.stack([res.results[2 * b]["Y"] + res.results[2 * b + 1]["Y"]
                  for b in range(B)])
    return y.astype(np.float32)


# revision 4
# speedup vs baseline: 1.0310x; 1.0310x over previous
"""Trainium2 Bass kernel for nn_Attention (B=4, N=2048, C=1024, H=16, D=64).

v2 — restructured from baseline for engine balance / overlap:
  - qkv q/k projection: single PSUM pass (8-kt accumulation) per
    [128,512] token chunk; bias fused in the DVE drain (tensor_scalar).
  - Dedicated PSUM pools: sc (2x[128,1024] = 4 banks, double-buffered
    scores), po (2x[65,512] = 2 banks, PV accumulators), work
    (2x[128,512] = 2 banks, everything else).
  - Pipelined emission: q-proj(p+1)@ic0, k-proj(p+1)@ic1, stats(p+1)@ic2
    inside attention(p); attention_end(p) at (p+1, ic0).
  - Pair 3: per-ic attention_end + interleaved output projection.
  - sq (squares for RMSNorm stats) on the idle Pool/GpSimd engine.
  - DMA order: wq/wk + xtb first; WP last.

Sharding: 8 cores; core c handles batch b=c//2 and heads [8*(c%2), ...).
Host sums the two partial proj outputs per batch.
"""

import os
import numpy as np
import ml_dtypes

B, N, C, H, D = 4, 2048, 1024, 16, 64
NCORES = 8
HPC = 8           # heads per core
CH = HPC * D      # 512 channels per core
VSEG = 2 * D + 4  # 132 cols per pair in v_aug: [64 v | 1 | 1][64 v | 1 | 1]
VW = 4 * VSEG     # 528
EPS = 1e-6

_CACHE = {}
LAST_RESULT = [None]


def _round_f32r(x):
    x = np.ascontiguousarray(x, dtype=np.float32)
    u = x.view(np.uint32)
    keep = np.uint32(0xFFFFF000)
    half = np.uint32(0x800)
    lsb = (u >> np.uint32(12)) & np.uint32(1)
    r = (u + (half - np.uint32(1)) + lsb) & keep
    return r.view(np.float32)


def _build_nc():
    import concourse.tile as tile
    import concourse.mybir as mybir
    from concourse import bacc

    F32 = mybir.dt.float32
    F32R = mybir.dt.float32r
    BF16 = mybir.dt.bfloat16
    AF = mybir.ActivationFunctionType

    nc = bacc.Bacc("TRN2", target_bir_lowering=False, debug=False,
                   num_devices=NCORES)

    XTB = nc.dram_tensor("XTB", [C, N], BF16, kind="ExternalInput")
    WQ = nc.dram_tensor("WQ", [C, CH], BF16, kind="ExternalInput")
    WK = nc.dram_tensor("WK", [C, CH], BF16, kind="ExternalInput")
    WVA = nc.dram_tensor("WVA", [C, VW], BF16, kind="ExternalInput")
    WP = nc.dram_tensor("WP", [CH, C], F32R, kind="ExternalInput")
    BQK = nc.dram_tensor("BQK", [128, 8], F32, kind="ExternalInput")
    BVA = nc.dram_tensor("BVA", [128, VW], F32, kind="ExternalInput")
    BP = nc.dram_tensor("BP", [128, C], F32, kind="ExternalInput")
    QKN = nc.dram_tensor("QKN", [128, 2], F32, kind="ExternalInput")
    BLK = nc.dram_tensor("BLK", [128, 192], F32R, kind="ExternalInput")
    SEL = nc.dram_tensor("SEL", [96, 256], F32R, kind="ExternalInput")
    ONESB = nc.dram_tensor("ONESB", [65, 64], F32R, kind="ExternalInput")
    Y = nc.dram_tensor("Y", [N, C], F32, kind="ExternalOutput")

    NT = N // 128          # 16 token tiles
    KT = C // 128          # 8 contraction tiles
    NCHUNK = N // 512      # 4 chunks of 512 tokens

    with tile.TileContext(nc) as tc:
        from contextlib import ExitStack
        with ExitStack() as ctx:
            const_p = ctx.enter_context(tc.tile_pool(name="const", bufs=1))
            xtb_p = ctx.enter_context(tc.tile_pool(name="xtb", bufs=8))
            outT_p = ctx.enter_context(tc.tile_pool(name="outT", bufs=4))

            # PSUM pools: sc 4 banks, po 2 banks, work 2 banks
            sc_ps = ctx.enter_context(
                tc.tile_pool(name="sc", bufs=2, space="PSUM"))
            po_ps = ctx.enter_context(
                tc.tile_pool(name="po", bufs=2, space="PSUM"))
            work_ps = ctx.enter_context(
                tc.tile_pool(name="work", bufs=2, space="PSUM"))

            # ---- weights for pair 0 first, then x, then constants ----
            pair_ctx = ExitStack()
            w_p = ctx.enter_context(tc.tile_pool(name="w", bufs=2))
            state = {}

            def emit_w_loads(hp, with_xtb=False):
                wq_sb = w_p.tile([128, C], BF16, tag="wq")
                wk_sb = w_p.tile([128, C], BF16, tag="wk")
                wq3 = WQ.ap().rearrange("(kt p) j -> p kt j", p=128)
                wk3 = WK.ap().rearrange("(kt p) j -> p kt j", p=128)
                hsl = slice(hp * 128, (hp + 1) * 128)
                if with_xtb:
                    # two DMA queues: evens+wk on SP, wq+odds on ACT, so
                    # the first qkv matmul's deps (wq, xtb0) land first.
                    nc.scalar.dma_start(wq_sb[:], wq3[:, :, hsl])
                    for kt in range(KT):
                        eng = nc.sync if kt % 2 == 0 else nc.scalar
                        eng.dma_start(
                            xtb_tiles[kt][:],
                            XTB.ap()[kt * 128:(kt + 1) * 128, :])
                    nc.sync.dma_start(wk_sb[:], wk3[:, :, hsl])
                else:
                    nc.sync.dma_start(wq_sb[:], wq3[:, :, hsl])
                    nc.sync.dma_start(wk_sb[:], wk3[:, :, hsl])
                st = state[hp] = {}
                st["wq"], st["wk"] = wq_sb, wk_sb
                st["qT_b"] = qtb_p.tile([128, N], F32, tag="qtb",
                                        name=f"qTb{hp}")
                st["kT_b"] = qtb_p.tile([128, N], F32, tag="ktb",
                                        name=f"kTb{hp}")

            qtb_p = ctx.enter_context(tc.tile_pool(name="qtb", bufs=1))

            xtb_tiles = []
            for kt in range(KT):
                xtb_tiles.append(
                    xtb_p.tile([128, N], BF16, tag="xtb", name=f"xtb{kt}"))
            emit_w_loads(0, with_xtb=True)

            # constants
            bqk_sb = const_p.tile([128, 8], F32, tag="bqk")
            nc.sync.dma_start(bqk_sb[:], BQK.ap()[:, :])
            bva_sb = const_p.tile([128, VW], F32, tag="bva")
            nc.sync.dma_start(bva_sb[:], BVA.ap()[:, :])
            qkn_sb = const_p.tile([128, 2], F32, tag="qkn")
            nc.sync.dma_start(qkn_sb[:], QKN.ap()[:, :])
            blk_sb = const_p.tile([128, 192], F32R, tag="blk")
            nc.sync.dma_start(blk_sb[:], BLK.ap()[:, :])
            sel_sb = const_p.tile([96, 256], F32R, tag="sel")
            nc.sync.dma_start(sel_sb[:], SEL.ap()[:, :])
            onesb_sb = const_p.tile([65, 64], F32R, tag="onesb")
            nc.sync.dma_start(onesb_sb[:], ONESB.ap()[:, :])
            eps_qk = const_p.tile([128, 1], F32, tag="eps_qk")
            nc.vector.memset(eps_qk[0:64, :], float(EPS))
            nc.vector.memset(eps_qk[64:128, :], float(EPS * 64))

            # ---------------- qkv q/k projection ------------------------
            def emit_qk_proj(hp, which, chunks=None):
                # which: 0 = q, 1 = k; single psum pass over all 8 kt
                st = state[hp]
                wsb = st["wk"] if which else st["wq"]
                dst = st["kT_b"] if which else st["qT_b"]
                bcol = (4 + hp) if which else hp
                for chk in (chunks if chunks is not None
                            else range(NCHUNK)):
                    csl = slice(chk * 512, (chk + 1) * 512)
                    ps = work_ps.tile([128, 512], F32, tag="work")
                    for kt in range(KT):
                        nc.tensor.matmul(
                            ps[:], wsb[:, kt * 128:(kt + 1) * 128],
                            xtb_tiles[kt][:, csl],
                            start=(kt == 0), stop=(kt == KT - 1))
                    nc.vector.tensor_scalar(
                        dst[:, csl], ps[:], bqk_sb[:, bcol:bcol + 1],
                        None, op0=mybir.AluOpType.add)

            # ---------------- stats (RMSNorm rsqrt) ---------------------
            def emit_stats_front(hp, chunks=None):
                # q var lands at rows 0/32, k sumsq at rows 64/96 of one
                # packed [97, N] tile (two matmuls, second accumulates into
                # the same psum bank at col position 64).  rsqrt is computed
                # without Ln (avoids ACT table switching amid the exps):
                # magic-log bit trick seeds x0 ~ ln(v), two Newton steps
                # x <- x - 1 + v*exp(-x) refine it on DVE+ACT(Exp), then
                # rs = exp(-0.5 x).
                st = state[hp]
                I32 = mybir.dt.int32
                if "rs" not in st:
                    st["rs"] = stat_p.tile([96, N], F32R, tag="rs",
                                           name=f"rs{hp}")
                rs = st["rs"]
                for chk in (chunks if chunks is not None
                            else range(NCHUNK)):
                    csl = slice(chk * 512, (chk + 1) * 512)
                    sq_q = sq_p.tile([128, 512], F32R, tag="sq")
                    nc.gpsimd.tensor_mul(sq_q[:], st["qT_b"][:, csl],
                                         st["qT_b"][:, csl])
                    sq_k = sq_p.tile([128, 512], F32R, tag="sq")
                    nc.gpsimd.tensor_mul(sq_k[:], st["kT_b"][:, csl],
                                         st["kT_b"][:, csl])
                    vps = work_ps.tile([96, 512], F32, tag="work")
                    nc.tensor.matmul(vps[:, :], blk_sb[:, 0:96], sq_q[:],
                                     start=True, stop=False)
                    nc.tensor.matmul(vps[:, :], blk_sb[:, 96:192],
                                     sq_k[:], start=False, stop=True)
                    vv = stat_p.tile([96, 512], F32, tag="vv")
                    nc.vector.tensor_scalar(vv[:], vps[:],
                                            eps_qk[0:96, :], None,
                                            op0=mybir.AluOpType.add)
                    x0 = stat_p.tile([96, 512], BF16, tag="x0")
                    nc.vector.tensor_scalar(
                        x0[:], vv[:].bitcast(I32), 8.2629582e-8,
                        -87.989971088, op0=mybir.AluOpType.mult,
                        op1=mybir.AluOpType.add)
                    e0 = stat_p.tile([96, 512], F32, tag="e")
                    nc.scalar.activation(e0[:], x0[:], AF.Exp, scale=-1.0)
                    t0 = stat_p.tile([96, 512], F32, tag="t")
                    nc.vector.tensor_mul(t0[:], vv[:], e0[:])
                    x1 = stat_p.tile([96, 512], F32, tag="xf", bufs=2)
                    nc.vector.scalar_tensor_tensor(
                        x1[:], t0[:], -1.0, x0[:],
                        op0=mybir.AluOpType.add, op1=mybir.AluOpType.add)
                    e1 = stat_p.tile([96, 512], F32, tag="e")
                    nc.scalar.activation(e1[:], x1[:], AF.Exp, scale=-1.0)
                    t1 = stat_p.tile([96, 512], F32, tag="t")
                    nc.vector.tensor_mul(t1[:], vv[:], e1[:])
                    x2 = stat_p.tile([96, 512], F32, tag="xf", bufs=2)
                    nc.vector.scalar_tensor_tensor(
                        x2[:], t1[:], -1.0, x1[:],
                        op0=mybir.AluOpType.add, op1=mybir.AluOpType.add)
                    nc.scalar.activation(rs[:, csl], x2[:], AF.Exp,
                                         scale=-0.5)

            def emit_stats_apply(hp, which=None):
                st = state[hp]
                rs = st["rs"]
                if "qTn" not in st:
                    st["qTn"] = qtn_p.tile([128, N], BF16, tag="qtn",
                                           name=f"qTn{hp}")
                    st["kTn"] = qtn_p.tile([128, N], BF16, tag="ktn",
                                           name=f"kTn{hp}")
                todo = ((0, 1) if which is None else (which,))
                for w in todo:
                    src_t = st["kT_b"] if w else st["qT_b"]
                    dstn = st["kTn"] if w else st["qTn"]
                    selsl = slice(128, 256) if w else slice(0, 128)
                    for chk in range(NCHUNK):
                        csl = slice(chk * 512, (chk + 1) * 512)
                        bc_ps = work_ps.tile([128, 512], F32, tag="work",
                                             name="bc_ps")
                        nc.tensor.matmul(
                            bc_ps[:], sel_sb[:, selsl], rs[:, csl],
                            start=True, stop=True)
                        nc.vector.scalar_tensor_tensor(
                            dstn[:, csl], src_t[:, csl],
                            qkn_sb[:, w:w + 1], bc_ps[:],
                            op0=mybir.AluOpType.mult,
                            op1=mybir.AluOpType.mult)

            # ---------------- v projection ------------------------------
            v_ctx = tc.tile_pool(name="v", bufs=16)
            v_p = v_ctx.__enter__()
            v_tiles = []
            for nt in range(NT):
                v_tiles.append(
                    v_p.tile([128, VW], BF16, tag="v", name=f"vt{nt}"))

            def emit_v_block(nt_range):
                for nt in nt_range:
                    for vh in range(4):
                        sl = slice(vh * (VW // 4), (vh + 1) * (VW // 4))
                        ps = work_ps.tile([128, VW // 4], F32, tag="work")
                        for kt in range(KT):
                            nc.tensor.matmul(
                                ps[:],
                                xtb_tiles[kt][:, nt * 128:(nt + 1) * 128],
                                wv_sb[:, kt * VW + vh * (VW // 4):
                                      kt * VW + (vh + 1) * (VW // 4)],
                                start=(kt == 0), stop=(kt == KT - 1))
                        nc.vector.tensor_add(
                            v_tiles[nt][:, sl], ps[:], bva_sb[:, sl])

            # ---------------- attention -------------------------------
            outT_tiles = []

            def emit_attention_start(hp):
                st = state[hp]
                outT = outT_p.tile([128, N], F32R, tag="outT",
                                   name=f"outT{hp}")
                outT_tiles.append(outT)
                st["outT"] = outT
                if hp < 3:
                    st["den"] = rcp_p.tile([8, 512], F32, tag="den_pack",
                                           bufs=1, name=f"den{hp}")
                st["po_sbs"] = []

            # int16 magic-exp constants: exp(s) ~ bf16_bits(s*A2 + B2);
            # B2 midpoint is robust to floor-vs-round f32->i16 conversion.
            MEXP_A = 128.0 / float(np.log(2.0))
            MEXP_B = 16250.75
            DVE_EXP_JTS = (7, 15)

            def emit_attention_ic(hp, ic, fillers=()):
                # fillers: up to 4 emit-thunks interleaved into the jt
                # stream (at jt 4/8/12 and after the last jt) so PE work
                # like qkv/stats/proj overlaps the ACT-paced exp stream.
                st = state[hp]
                qTn, kTn, outT = st["qTn"], st["kTn"], st["outT"]
                I16 = mybir.dt.int16
                vbase = hp * VSEG
                isl = slice(ic * 512, (ic + 1) * 512)
                poA = po_ps.tile([65, 512], F32, tag="po")
                poB = po_ps.tile([65, 512], F32, tag="po")
                for jt in range(NT):
                    if jt in (4, 8, 12) and len(fillers) > jt // 4 - 1:
                        fillers[jt // 4 - 1]()
                    jsl = slice(jt * 128, (jt + 1) * 128)
                    sc = sc_ps.tile([128, 1024], F32, tag="sc")
                    nc.tensor.matmul(
                        sc[:, 0:512], kTn[0:64, jsl], qTn[0:64, isl],
                        start=True, stop=True, tile_position=(0, 0))
                    nc.tensor.matmul(
                        sc[:, 512:1024], kTn[64:128, jsl], qTn[64:128, isl],
                        start=True, stop=True, tile_position=(64, 0))
                    if jt in DVE_EXP_JTS:
                        exi = ex_p.tile([128, 1024], I16, tag="ex")
                        with nc.allow_low_precision(
                                reason="magic-exp approx on DVE"):
                            nc.vector.tensor_scalar(
                                exi[:], sc[:], MEXP_A, MEXP_B,
                                op0=mybir.AluOpType.mult,
                                op1=mybir.AluOpType.add)
                        ex0 = exi[:, 0:512].bitcast(BF16)
                        ex1 = exi[:, 512:1024].bitcast(BF16)
                    else:
                        ext = ex_p.tile([128, 1024], BF16, tag="ex")
                        nc.scalar.activation(ext[:], sc[:], AF.Exp)
                        ex0 = ext[:, 0:512]
                        ex1 = ext[:, 512:1024]
                    nc.tensor.matmul(
                        poA[:], v_tiles[jt][:, vbase:vbase + 65],
                        ex0, start=(jt == 0), stop=(jt == NT - 1))
                    nc.tensor.matmul(
                        poB[:],
                        v_tiles[jt][:, vbase + VSEG // 2:
                                    vbase + VSEG // 2 + 65],
                        ex1, start=(jt == 0),
                        stop=(jt == NT - 1))
                if len(fillers) > 3:
                    fillers[3]()
                for hh, (po, rowoff) in enumerate(((poA, 0), (poB, 64))):
                    idx = ic * 2 + hh
                    po_sb = rcp_p.tile([65, 512], F32, tag="po_sb",
                                       name=f"po_sb{hp}_{idx}", bufs=10)
                    nc.vector.tensor_copy(po_sb[:], po[:, :])
                    if hp < 3:
                        nc.sync.dma_start(st["den"][idx:idx + 1, :],
                                          po_sb[64:65, :])
                    st["po_sbs"].append((po_sb, rowoff, ic, idx))

            def emit_attention_end(hp):
                # pairs 0-2: batched rcp for all 4 ics
                st = state[hp]
                outT = st["outT"]
                rcp_pack = rcp_p.tile([8, 512], F32R, tag="rcp_pack",
                                      bufs=1, name=f"rcpp{hp}")
                with nc.allow_low_precision(
                        reason="softmax denom recip rounded to f32r"):
                    nc.vector.reciprocal(rcp_pack[:], st["den"][:])
                rcp_al = rcp_p.tile([65, 1536], F32R, tag="rcp_al",
                                    bufs=1, name=f"rcpa{hp}")
                for idx in range(8):
                    r, fb = 32 * (idx % 3), 512 * (idx // 3)
                    nc.sync.dma_start(rcp_al[r:r + 1, fb:fb + 512],
                                      rcp_pack[idx:idx + 1, :])
                for (po_sb, rowoff, ic, idx) in st["po_sbs"]:
                    isl = slice(ic * 512, (ic + 1) * 512)
                    r, fb = 32 * (idx % 3), 512 * (idx // 3)
                    rb_ps = work_ps.tile([64, 512], F32, tag="work",
                                         name="rb_ps")
                    nc.tensor.matmul(rb_ps[:], onesb_sb[r:r + 1, :],
                                     rcp_al[r:r + 1, fb:fb + 512],
                                     start=True, stop=True)
                    nc.vector.tensor_mul(
                        outT[rowoff:rowoff + 64, isl], po_sb[0:64, :],
                        rb_ps[:])

            def emit_attention_end3_rcp(ic):
                # pair 3, one ic: rcp + outT for its two po tiles. The
                # reciprocal runs in place at partition 64 (same partition
                # as po_sb's denominator row and onesb's ones row), so no
                # cross-partition DMA sits on the critical path.
                st = state[3]
                outT = st["outT"]
                rcp3 = rcp_p.tile([65, 1024], F32R, tag="rcp3", bufs=1,
                                  name=f"rcp3_{ic}")
                ents = [e for e in st["po_sbs"] if e[2] == ic]
                isl = slice(ic * 512, (ic + 1) * 512)
                for (po_sb, rowoff, _ic, idx) in ents:
                    fb = 512 * (idx % 2)
                    with nc.allow_low_precision(
                            reason="softmax denom recip rounded to f32r"):
                        nc.vector.reciprocal(rcp3[64:65, fb:fb + 512],
                                             po_sb[64:65, :])
                for (po_sb, rowoff, _ic, idx) in ents:
                    fb = 512 * (idx % 2)
                    rb_ps = work_ps.tile([64, 512], F32, tag="work",
                                         name="rb_ps")
                    nc.tensor.matmul(rb_ps[:], onesb_sb[64:65, :],
                                     rcp3[64:65, fb:fb + 512],
                                     start=True, stop=True)
                    nc.vector.tensor_mul(
                        outT[rowoff:rowoff + 64, isl], po_sb[0:64, :],
                        rb_ps[:])

            # ---------------- output projection -------------------------
            def emit_proj_nt(nt):
                nsl = slice(nt * 128, (nt + 1) * 128)
                for sub in range(2):
                    ssl = slice(sub * 512, (sub + 1) * 512)
                    ps = work_ps.tile([128, 512], F32, tag="work")
                    for kt in range(4):
                        nc.tensor.matmul(
                            ps[:], outT_tiles[kt][:, nsl],
                            wp_tiles[kt][:, ssl],
                            start=(kt == 0), stop=(kt == 3))
                    y_sb = y_p.tile([128, 512], F32, tag="y")
                    nc.vector.tensor_add(y_sb[:], ps[:], bp_sb[:, ssl])
                    nc.sync.dma_start(Y.ap()[nsl, ssl], y_sb[:])

            # ---------------- emission schedule -------------------------
            sq_p = pair_ctx.enter_context(tc.tile_pool(name="sq", bufs=1))
            stat_p = pair_ctx.enter_context(tc.tile_pool(name="stat",
                                                         bufs=1))
            qtn_p = pair_ctx.enter_context(tc.tile_pool(name="qtn", bufs=2))
            rcp_p = pair_ctx.enter_context(tc.tile_pool(name="rcp", bufs=1))
            ex_p = pair_ctx.enter_context(tc.tile_pool(name="ex", bufs=2))
            bp_sb = const_p.tile([128, C], F32, tag="bp")
            nc.sync.dma_start(bp_sb[:], BP.ap()[:, :])

            wv_ctx = tc.tile_pool(name="wv", bufs=1)
            wv_p = wv_ctx.__enter__()
            wv_sb = wv_p.tile([128, KT * VW], BF16, tag="wv")
            nc.sync.dma_start(
                wv_sb[:],
                WVA.ap().rearrange("(kt p) j -> p kt j", p=128))

            for chk in range(NCHUNK):
                emit_qk_proj(0, 0, [chk])
                emit_qk_proj(0, 1, [chk])
                emit_stats_front(0, [chk])
                emit_v_block(range(4 * chk, 4 * chk + 4))
            emit_stats_apply(0)
            wv_ctx.__exit__(None, None, None)

            wp_tiles = []

            def emit_wp_loads():
                wp_p = pair_ctx.enter_context(tc.tile_pool(name="wp",
                                                           bufs=4))
                for kt in range(4):
                    t = wp_p.tile([128, C], F32R, tag="wp")
                    nc.sync.dma_start(t[:], WP.ap()[kt * 128:(kt + 1) * 128, :])
                    wp_tiles.append(t)

            for hp in range(4):
                emit_attention_start(hp)
                if hp + 1 < 4:
                    emit_w_loads(hp + 1)
                if hp == 2:
                    emit_wp_loads()
                    y_p = pair_ctx.enter_context(tc.tile_pool(name="y",
                                                              bufs=2))
                for ic in range(NCHUNK):
                    if hp + 1 < 4:
                        nx = hp + 1
                        if ic == 0:
                            fillers = [
                                (lambda c=c, nx=nx:
                                 emit_qk_proj(nx, 0, [c]))
                                for c in range(4)]
                            if hp > 0:
                                fillers[3] = (
                                    lambda f=fillers[3], p=hp - 1:
                                    (f(), emit_attention_end(p)))
                        elif ic == 1:
                            fillers = [
                                (lambda c=c, nx=nx:
                                 (emit_qk_proj(nx, 1, [c]),
                                  emit_stats_front(nx, [c])))
                                for c in range(4)]
                        elif ic == 2:
                            noop = lambda: None
                            fillers = [
                                (lambda nx=nx: emit_stats_apply(nx, 0)),
                                (lambda nx=nx: emit_stats_apply(nx, 1)),
                                noop, noop]
                        else:
                            fillers = []
                    else:
                        if ic == 0:
                            fillers = [lambda: emit_attention_end(2)]
                        else:
                            pic = ic - 1
                            fillers = [
                                (lambda p=pic:
                                 emit_attention_end3_rcp(p)),
                                (lambda p=pic: emit_proj_nt(4 * p)),
                                (lambda p=pic: (emit_proj_nt(4 * p + 1),
                                                emit_proj_nt(4 * p + 2))),
                                (lambda p=pic: emit_proj_nt(4 * p + 3))]
                    emit_attention_ic(hp, ic, fillers)
            emit_attention_end3_rcp(3)
            for nt in range(12, 16):
                emit_proj_nt(nt)

            pair_ctx.close()
            v_ctx.__exit__(None, None, None)

    nc.compile()
    return nc


def _core_inputs(c, x, W_qkv, b_qkv, W_proj, b_proj, qn_w, kn_w):
    b, half = c // 2, c % 2
    hbase = HPC * half
    co = hbase * D                      # channel offset of this core's heads

    xT = np.ascontiguousarray(x[b].T, dtype=np.float32)
    WQc = W_qkv[:, co:co + CH].astype(ml_dtypes.bfloat16)
    WKc = W_qkv[:, C + co:C + co + CH].astype(ml_dtypes.bfloat16)
    WVc = W_qkv[:, 2 * C + co:2 * C + co + CH]
    WVA = np.zeros((C, VW), dtype=np.float32)
    BVA1 = np.zeros((VW,), dtype=np.float32)
    bv = b_qkv[2 * C + co:2 * C + co + CH]
    for hp in range(4):
        for hh in range(2):
            s = hp * VSEG + hh * (VSEG // 2)
            WVA[:, s:s + D] = WVc[:, (2 * hp + hh) * D:(2 * hp + hh + 1) * D]
            BVA1[s:s + D] = bv[(2 * hp + hh) * D:(2 * hp + hh + 1) * D]
            BVA1[s + D] = 1.0  # ones column for softmax denominators
    WVA = WVA.astype(ml_dtypes.bfloat16)
    BVA = np.broadcast_to(BVA1, (128, VW)).copy()

    BQK = np.zeros((128, 8), dtype=np.float32)
    for hp in range(4):
        BQK[:, hp] = b_qkv[co + hp * 128:co + (hp + 1) * 128]
        BQK[:, 4 + hp] = b_qkv[C + co + hp * 128:C + co + (hp + 1) * 128]

    WPc = _round_f32r(W_proj[co:co + CH, :])
    BP = (np.broadcast_to(b_proj, (128, C)).copy() if half == 0
          else np.zeros((128, C), dtype=np.float32))
    QKN = np.stack([np.tile(qn_w, 2), np.tile(kn_w, 2)], axis=1).astype(np.float32)
    BLK = np.zeros((128, 192), dtype=np.float32)
    BLK[0:64, 0] = 1.0 / D        # q head0 -> var row 0
    BLK[64:128, 32] = 1.0 / D     # q head1 -> var row 32
    BLK[0:64, 96 + 64] = 1.0      # k head0 -> row 64 (1/8 folds into rsqrt)
    BLK[64:128, 96 + 95] = 1.0    # k head1 -> row 95
    BLK = _round_f32r(BLK)
    SEL = np.zeros((96, 256), dtype=np.float32)
    SEL[0, 0:64] = 1.0      # q head0 -> channels 0-63
    SEL[32, 64:128] = 1.0   # q head1
    SEL[64, 128:192] = 1.0  # k head0
    SEL[95, 192:256] = 1.0  # k head1
    SEL = _round_f32r(SEL)
    ONESB = np.zeros((65, 64), dtype=np.float32)
    for r in (0, 32, 64):
        ONESB[r, :] = 1.0
    ONESB = _round_f32r(ONESB)

    xTb = xT.astype(ml_dtypes.bfloat16)
    return {"XTB": xTb, "WQ": WQc, "WK": WKc, "WVA": WVA, "WP": WPc,
            "BQK": BQK, "BVA": BVA, "BP": BP.astype(np.float32),
            "QKN": QKN, "BLK": BLK, "SEL": SEL, "ONESB": ONESB}


def kernel(x, W_qkv, b_qkv, W_proj, b_proj, qn_w, kn_w):
    from concourse.bass_utils import run_bass_kernel_spmd

    if "nc" not in _CACHE:
        _CACHE["nc"] = _build_nc()
    nc = _CACHE["nc"]

    args = (np.asarray(x, np.float32), np.asarray(W_qkv, np.float32),
            np.asarray(b_qkv, np.float32), np.asarray(W_proj, np.float32),
            np.asarray(b_proj, np.float32), np.asarray(qn_w, np.float32),
            np.asarray(kn_w, np.float32))
    in_maps = [_core_inputs(c, *args) for c in range(NCORES)]

    trace = os.environ.get("BASS_KERNEL_TRACE", "0") == "1"
    res = run_bass_kernel_spmd(nc, in_maps, core_ids=list(range(NCORES)),
                               trace=trace)
    LAST_RESULT[0] = res

    y = np.stack([res.results[2 * b]["Y"] + res.results[2 * b + 1]["Y"]
                  for b in range(B)])
    return y.astype(np.float32)
